# revision 1
# baseline (speedup 1.0000x reference)
"""Causal multi-head attention (QKV-packed) on 8 Trainium2 NeuronCores.

Sharding: pure head-parallel. B*H = 32 (batch, head) pairs -> 4 per core,
zero inter-core communication. Per head, flash-style causal attention is
computed entirely in the "transposed" orientation so no on-device
transposes are needed:

  - Host pre-lays-out Q^T, K^T as [D=128, S] (D on partitions) and V as
    k-blocks [128, D]; scores are computed transposed S_T[k, q] =
    (K^T_j).T @ Q^T, softmax numerator P_T = exp(scale * S_T + mask) on
    the ACT engine, then O^T[d, q] += V_j.T @ P_T accumulates in PSUM.
    The softmax denominator comes from a ones-vector matmul over P_T,
    also PSUM-accumulated; normalization is a K=1 broadcast matmul of
    the reciprocal plus one DVE multiply. Output is returned as O^T and
    un-transposed on the host.
  - All matmuls run as float32r (full-rate fp32 at free-dim >= 256).
  - exp() skips max-subtraction: scores are ~N(0,1) after 1/sqrt(D)
    scaling, so exp is safely in fp32 range.
"""

import sys

if "/opt/trn_rl_repo" not in sys.path:
    sys.path.insert(0, "/opt/trn_rl_repo")

import numpy as np

B, S, H, D = 2, 2048, 16, 128
NCORES = 8
HPC = (B * H) // NCORES  # heads per core = 4
QS = 512   # q-strip width (PSUM bank)
KB = 128   # k-block (partition dim)
NEG = -1.0e30
SCALE = 1.0 / float(np.sqrt(D))
NSTRIP = S // QS  # 4

_nc_cache = {}


def _block_geometry(s, j):
    """For q-strip s and k-block j return (off, N): the strip-local column
    range [off, off+N) of q positions this block contributes to.
    t = j - 4s is the diagonal offset; t<0 full block, t>=0 diagonal."""
    t = j - 4 * s
    if t <= 0:
        return 0, QS
    if t == 1:
        return 128, 384
    # t == 2 and t == 3 both use 256 columns (t=3 widened so the fp32r
    # matmul keeps free-dim >= 256; its extra 128 columns are fully masked)
    return 256, 256


def _build_nc():
    import concourse.bass as bass  # noqa: F401
    import concourse.mybir as mybir
    from concourse import bacc
    from concourse.tile import TileContext

    f32 = mybir.dt.float32
    f32r = mybir.dt.float32r
    Exp = mybir.ActivationFunctionType.Exp

    nc = bacc.Bacc()
    # One packed input per head (single DMA => single wait semaphore for
    # the first matmul of each head; walrus allows only one sync-wait on
    # an fp32r matmul's fused weight load). Layout per head [128, 3*S]:
    # cols [0,S) = Q^T, [S,2S) = K^T, [2S,3S) = V swizzled so column
    # block j holds the V k-block [128, D] (v[p, j*KB+d] = V[j*KB+p, d]).
    qkvT = nc.declare_dram_parameter("qkvT", [HPC, 128, 3 * S], f32r, isOutput=False)
    cst = nc.declare_dram_parameter("cst", [128, 256], f32, isOutput=False)
    ones = nc.declare_dram_parameter("ones", [128, 128], f32r, isOutput=False)
    oT = nc.declare_dram_parameter("oT", [HPC, 128, S], f32, isOutput=True)

    with TileContext(nc) as tc:
        with (
            nc.allow_low_precision(
                reason="float32r is 4-byte; reciprocal into f32r is fine"
            ),
            tc.tile_pool(name="cpool", bufs=1) as cpool,
            tc.tile_pool(name="qkpool", bufs=2) as qkpool,
            tc.tile_pool(name="ptpool", bufs=6) as ptpool,
            tc.tile_pool(name="obpool", bufs=2) as obpool,
            tc.tile_pool(name="pst", bufs=3, space="PSUM") as pst,
            tc.tile_pool(name="pso", bufs=2, space="PSUM") as pso,
            tc.tile_pool(name="psd", bufs=2, space="PSUM") as psd,
            tc.tile_pool(name="psr", bufs=1, space="PSUM") as psr,
        ):
            cst_sb = cpool.tile([128, 256], f32)
            nc.sync.dma_start(out=cst_sb[:], in_=cst[:])
            ones_sb = cpool.tile([128, 128], f32r)
            nc.sync.dma_start(out=ones_sb[:], in_=ones[:])
            tri = cst_sb[:, 0:128]       # tri[dk, c] = 0 if dk <= c else NEG
            full = cst_sb[:, 128:256]    # all NEG
            ones_col = ones_sb[:, 0:1]   # [128, 1] of 1.0
            ones_row = ones_sb[0:1, :]   # [1, 128] of 1.0

            def epilogue_rest(h, s, o_ps, recip):
                rb = psr.tile([128, QS], f32, tag="rb")
                nc.tensor.matmul(
                    rb[:], lhsT=ones_row, rhs=recip[:], start=True, stop=True
                )
                rb_sb = obpool.tile([128, QS], f32, tag="rb_sb")
                nc.vector.tensor_copy(rb_sb[:], rb[:])
                o_sb = obpool.tile([128, QS], f32, tag="o_sb")
                nc.vector.tensor_mul(o_sb[:], o_ps[:], rb_sb[:])
                nc.sync.dma_start(out=oT[h][:, QS * s : QS * (s + 1)], in_=o_sb[:])

            pending = None  # (h, s, o_ps, recip) of the previous strip
            for h in range(HPC):
                qkv_sb = qkpool.tile([128, 3 * S], f32r, tag="qkv_sb")
                if h == 0:
                    # split the first head's load so the first matmuls can
                    # start after ~0.5MB instead of the full 3MB: K^T and
                    # Q^T for strip 0 first, then V for strip 0, then rests
                    for c0, c1 in (
                        (S, S + 512),          # K^T blocks 0-3
                        (0, 512),              # Q^T strip 0
                        (2 * S, 2 * S + 512),  # V blocks 0-3
                        (512, S),              # Q^T rest
                        (S + 512, 2 * S),      # K^T rest
                        (2 * S + 512, 3 * S),  # V rest
                    ):
                        nc.sync.dma_start(
                            out=qkv_sb[:, c0:c1], in_=qkvT[h][:, c0:c1]
                        )
                else:
                    nc.sync.dma_start(out=qkv_sb[:], in_=qkvT[h])
                qt_sb = qkv_sb[:, 0:S]
                kt_sb = qkv_sb[:, S : 2 * S]
                v_sb = qkv_sb[:, 2 * S : 3 * S]

                for s in range(NSTRIP):
                    o_ps = pso.tile([128, QS], f32, tag="o_ps")
                    den_ps = psd.tile([1, QS], f32, tag="den_ps")
                    nblk = 4 * s + 4
                    for j in range(nblk):
                        t = j - 4 * s
                        off, N = _block_geometry(s, j)
                        sT = pst.tile([128, QS], f32, tag="sT")
                        nc.tensor.matmul(
                            sT[:, 0:N],
                            lhsT=kt_sb[:, KB * j : KB * (j + 1)],
                            rhs=qt_sb[:, QS * s + off : QS * s + off + N],
                            start=True,
                            stop=True,
                        )
                        if t >= 0:
                            if t == 3:
                                # strip cols [256,384) fully masked,
                                # [384,512) triangular
                                nc.vector.tensor_add(
                                    sT[:, 0:128], sT[:, 0:128], full
                                )
                                nc.vector.tensor_add(
                                    sT[:, 128:256], sT[:, 128:256], tri
                                )
                            else:
                                loc = 128 * t - off  # always 0 here
                                nc.vector.tensor_add(
                                    sT[:, loc : loc + 128], sT[:, loc : loc + 128], tri
                                )
                        pt = ptpool.tile([128, QS], f32r, tag="pt")
                        nc.scalar.activation(pt[:, 0:N], sT[:, 0:N], Exp, scale=SCALE)
                        first, last = (j == 0), (j == nblk - 1)
                        nc.tensor.matmul(
                            o_ps[:, off : off + N],
                            lhsT=v_sb[:, KB * j : KB * (j + 1)],
                            rhs=pt[:, 0:N],
                            start=first,
                            stop=last,
                        )
                        nc.tensor.matmul(
                            den_ps[:, off : off + N],
                            lhsT=ones_col,
                            rhs=pt[:, 0:N],
                            start=first,
                            stop=last,
                        )
                    # reciprocal immediately (den_ps has bufs=1; freeing it
                    # fast keeps the single den bank available), but the rest
                    # of the epilogue is software-pipelined behind the next
                    # strip's body so its PE/DVE work overlaps the matmuls
                    recip = obpool.tile([1, QS], f32r, tag="recip")
                    nc.vector.reciprocal(recip[:], den_ps[:])
                    if pending is not None:
                        epilogue_rest(*pending)
                    pending = (h, s, o_ps, recip)
            epilogue_rest(*pending)
    nc.compile()
    return nc


def get_nc():
    if "nc" not in _nc_cache:
        _nc_cache["nc"] = _build_nc()
    return _nc_cache["nc"]


def _build_const():
    dk = np.arange(128)[:, None]
    c = np.arange(128)[None, :]
    cst = np.empty((128, 256), np.float32)
    cst[:, 0:128] = np.where(dk <= c, 0.0, NEG).astype(np.float32)
    cst[:, 128:256] = NEG
    return cst


def make_in_maps(qkv):
    qkv = np.asarray(qkv, dtype=np.float32)
    cst = _build_const()
    in_maps = []
    for core in range(NCORES):
        qkvT = np.empty((HPC, 128, 3 * S), np.float32)
        for i in range(HPC):
            bh = core * HPC + i
            b, h = bh // H, bh % H
            qkvT[i, :, 0:S] = qkv[b, :, 0, h, :].T
            qkvT[i, :, S : 2 * S] = qkv[b, :, 1, h, :].T
            qkvT[i, :, 2 * S : 3 * S] = (
                qkv[b, :, 2, h, :]
                .reshape(S // KB, KB, D)
                .transpose(1, 0, 2)
                .reshape(KB, S)
            )
        in_maps.append({"qkvT": qkvT, "cst": cst,
                        "ones": np.ones((128, 128), np.float32)})
    return in_maps


def assemble_out(results):
    out = np.empty((B, S, H, D), np.float32)
    for core in range(NCORES):
        oTc = results[core]["oT"]  # [HPC, 128, S]
        for i in range(HPC):
            bh = core * HPC + i
            b, h = bh // H, bh % H
            out[b, :, h, :] = oTc[i].T
    return out


def kernel(qkv):
    from concourse.bass_utils import run_bass_kernel_spmd

    in_maps = make_in_maps(qkv)
    nc = get_nc()
    res = run_bass_kernel_spmd(nc, in_maps, list(range(NCORES)))
    return assemble_out(res.results)

